# revision 1
# baseline (speedup 1.0000x reference)
"""GroupedQueryAttention Trainium2 kernel.

Sharding: 8 cores = 2 (batch) x 4 (KV-head groups). Each core computes, for
its batch b and its 2 KV heads (8 query heads = 512 q dims):
  qT = (Wq_slice @ hidden[b].T + bq)      [512, S]   (d on partitions)
  kT = (Wk_slice @ hidden[b].T + bk)      [128, S]
  vT = ...                                 [128, S] -> PE-transposed to v [t, d]
  per head: scoresT[t,s] = k.q / sqrt(D); exp; PV via [v|1] matmul (Z in row 64)
  o_partial[s, :] = attn_norm[s, 512] @ Wo_slice  (row-parallel)
Host sums the 4 partials per batch and adds bo.

All matmuls run in float32r (TF32-like, ~1.6e-4 relerr, full PE rate).
"""

import numpy as np

import concourse.bass as bass
import concourse.mybir as mybir
import concourse.tile as tile
from concourse import bacc
from concourse.masks import make_identity
from concourse.bass_utils import run_bass_kernel_spmd

P = 128
B, S, HID = 2, 2048, 2048
NH, G = 32, 8
HG = NH // G            # 4 query heads per KV head
D = HID // NH           # 64
NCORES = 8
GS = NCORES // B        # 4 head-group shards
DQ = HID // GS          # 512 q dims per core
DKV = G * D // GS       # 128 kv dims per core
CH = 512                # s-chunk width
NCH = S // CH           # 4
KT = HID // P           # 16 contraction tiles for projections
TT = S // P             # 16 key tiles
NPAIR = DQ // P         # 4 head pairs per core
OKT = DQ // P           # 4 o-proj contraction tiles

f32 = mybir.dt.float32
f32r = mybir.dt.float32r
EXPF = mybir.ActivationFunctionType.Exp
SCALE = 1.0 / float(np.sqrt(D))


def _emit(tc):
    nc = tc.nc
    ht = nc.dram_tensor("ht", [HID, S], f32, kind="ExternalInput")
    wq = nc.dram_tensor("wq", [HID, DQ], f32, kind="ExternalInput")
    wk = nc.dram_tensor("wk", [HID, DKV], f32, kind="ExternalInput")
    wv = nc.dram_tensor("wv", [HID, DKV], f32, kind="ExternalInput")
    wo = nc.dram_tensor("wo", [DQ, HID], f32, kind="ExternalInput")
    bqd = nc.dram_tensor("bq", [DQ], f32, kind="ExternalInput")
    bkd = nc.dram_tensor("bk", [DKV], f32, kind="ExternalInput")
    bvd = nc.dram_tensor("bv", [DKV], f32, kind="ExternalInput")
    opart = nc.dram_tensor("opart", [S, HID], f32, kind="ExternalOutput")

    consts = tc.alloc_tile_pool(name="consts", bufs=1)
    wbig = tc.alloc_tile_pool(name="wbig", bufs=1)
    wkvp = tc.alloc_tile_pool(name="wkv", bufs=1)
    htp = tc.alloc_tile_pool(name="htp", bufs=10)
    persist = tc.alloc_tile_pool(name="persist", bufs=1)
    work = tc.alloc_tile_pool(name="work", bufs=2)
    expp = tc.alloc_tile_pool(name="expp", bufs=3)

    ones_d = nc.dram_tensor("ones", [P, D], f32, kind="ExternalInput")
    bsel_d = nc.dram_tensor("bsel", [33, P], f32, kind="ExternalInput")
    zeros_d = nc.dram_tensor("zeros", [33, CH], f32, kind="ExternalInput")

    ident = consts.tile([P, P], f32)
    make_identity(nc, ident)
    # selector matrix: row 0 -> partitions 0:64, row 32 -> partitions 64:128
    bsel = consts.tile([33, P], f32r)
    nc.sync.dma_start(out=bsel[:], in_=bsel_d[:].bitcast(f32r))
    # persistent zrec, zero-initialized once (rows 1..31 stay zero)
    zrec = consts.tile([33, CH], f32r)
    nc.sync.dma_start(out=zrec[:], in_=zeros_d[:].bitcast(f32r))
    bq_t = consts.tile([P, NPAIR], f32)
    nc.sync.dma_start(out=bq_t[:], in_=bqd.rearrange("(mt p) -> p mt", p=P))
    bk_t = consts.tile([P, 1], f32)
    nc.sync.dma_start(out=bk_t[:], in_=bkd.rearrange("(p one) -> p one", p=P))
    bv_t = consts.tile([P, 1], f32)
    nc.sync.dma_start(out=bv_t[:], in_=bvd.rearrange("(p one) -> p one", p=P))

    # weights (float32r via DMA bitcast); wq split so early k-tiles land fast
    wk_sb = wkvp.tile([P, KT, DKV], f32r)
    nc.sync.dma_start(out=wk_sb[:], in_=wk.rearrange("(kt p) m -> p kt m", p=P).bitcast(f32r))
    wv_sb = wkvp.tile([P, KT, DKV], f32r)
    nc.sync.dma_start(out=wv_sb[:], in_=wv.rearrange("(kt p) m -> p kt m", p=P).bitcast(f32r))
    wq_sb = wbig.tile([P, KT, DQ], f32r, tag="wbig")
    wq_r = wq.rearrange("(kt p) m -> p kt m", p=P).bitcast(f32r)
    for q4 in range(4):
        nc.sync.dma_start(out=wq_sb[:, 4 * q4:4 * (q4 + 1), :], in_=wq_r[:, 4 * q4:4 * (q4 + 1), :])

    qT_sb = persist.tile([P, NPAIR, S], f32r)
    ktrepA = persist.tile([P, S], f32r)
    ktrepB = persist.tile([P, S], f32r)
    vT_sb = persist.tile([P, S], f32)
    v_tiles = persist.tile([P, TT, 2, D + 1], f32r)
    attn_sb = persist.tile([P, OKT, S], f32r)

    # ---- pass 1: projections ----
    with tc.tile_pool(name="ps1", bufs=7, space="PSUM") as ps1:
        nc.sync.dma_start(out=v_tiles[:, :, :, D:D + 1], in_=ones_d[:, 0:TT * 2].bitcast(f32r))
        for c in range(NCH):
            cs = slice(c * CH, (c + 1) * CH)
            qps = [ps1.tile([P, CH], f32, tag="p1", name=f"qps{mt}") for mt in range(NPAIR)]
            kps = ps1.tile([P, CH], f32, tag="p1")
            vps = ps1.tile([P, CH], f32, tag="p1")
            for kt in range(KT):
                htt = htp.tile([P, CH], f32r)
                nc.sync.dma_start(out=htt[:], in_=ht[kt * P:(kt + 1) * P, cs].bitcast(f32r))
                for mt in range(NPAIR):
                    nc.tensor.matmul(qps[mt][:], wq_sb[:, kt, mt * P:(mt + 1) * P],
                                     htt[:], start=(kt == 0), stop=(kt == KT - 1))
                nc.tensor.matmul(kps[:], wk_sb[:, kt, :], htt[:],
                                 start=(kt == 0), stop=(kt == KT - 1))
                nc.tensor.matmul(vps[:], wv_sb[:, kt, :], htt[:],
                                 start=(kt == 0), stop=(kt == KT - 1))
            for mt in range(NPAIR):
                nc.vector.tensor_scalar_add(qT_sb[:, mt, cs], qps[mt][:], bq_t[:, mt:mt + 1])
            ktmp = work.tile([P, CH], f32r, tag="ktmp")
            nc.vector.tensor_scalar_add(ktmp[:], kps[:], bk_t[:, 0:1])
            nc.sync.dma_start(out=ktrepA[0:D, cs], in_=ktmp[0:D, :])
            nc.sync.dma_start(out=ktrepA[D:P, cs], in_=ktmp[0:D, :])
            nc.sync.dma_start(out=ktrepB[0:D, cs], in_=ktmp[D:P, :])
            nc.sync.dma_start(out=ktrepB[D:P, cs], in_=ktmp[D:P, :])
            nc.vector.tensor_scalar_add(vT_sb[:, cs], vps[:], bv_t[:, 0:1])
            # transpose this chunk's v: vT [dkv, t] -> v_tiles [t, dkv]
            for i in range(4 * c, 4 * (c + 1)):
                tp = ps1.tile([P, P], f32, tag="p1")
                nc.tensor.transpose(tp[:], vT_sb[:, i * P:(i + 1) * P], ident[:])
                for g in range(2):
                    nc.vector.tensor_copy(v_tiles[:, i, g, 0:D], tp[:, g * D:(g + 1) * D])

    # ---- pass 2: attention + interleaved o-projection ----
    # wo reuses wq's SBUF slot (same tag); its DMA starts once pass 1 releases wq
    wo_sb = wbig.tile([P, OKT, HID], f32r, tag="wbig")
    nc.sync.dma_start(out=wo_sb[:], in_=wo.rearrange("(kt p) m -> p kt m", p=P).bitcast(f32r))
    with tc.tile_pool(name="ps2", bufs=1, space="PSUM") as ps2:
        def emit_oproj(st):
            ss = slice(st * P, (st + 1) * P)
            for hc in range(HID // CH):
                hs = slice(hc * CH, (hc + 1) * CH)
                op = ps2.tile([P, CH], f32, tag="aux", bufs=2, name="op")
                for kt in range(OKT):
                    nc.tensor.matmul(op[:], attn_sb[:, kt, ss], wo_sb[:, kt, hs],
                                     start=(kt == 0), stop=(kt == OKT - 1))
                ostg = work.tile([P, CH], f32, tag="ostg", bufs=4, name="ostg")
                nc.vector.tensor_copy(ostg[:], op[:])
                nc.sync.dma_start(out=opart[ss, hs], in_=ostg[:])

        for c in range(NCH):
            cs = slice(c * CH, (c + 1) * CH)
            for p in range(NPAIR):
                ktrep = ktrepA if p < 2 else ktrepB
                g = p // 2
                pvE = ps2.tile([D + 1, CH], f32, tag="pv", bufs=2)
                pvO = ps2.tile([D + 1, CH], f32, tag="pv", bufs=2)
                for t in range(TT):
                    sc = ps2.tile([P, 2, CH], f32, tag="sc", bufs=2)
                    ts_ = slice(t * P, (t + 1) * P)
                    nc.tensor.matmul(sc[:, 0, :], ktrep[0:D, ts_], qT_sb[0:D, p, cs],
                                     tile_position=(0, 0), start=True, stop=True)
                    nc.tensor.matmul(sc[:, 1, :], ktrep[D:P, ts_], qT_sb[D:P, p, cs],
                                     tile_position=(D, 0), start=True, stop=True)
                    ex = expp.tile([P, 2, CH], f32r, tag="exp")
                    nc.scalar.activation(out=ex[:], in_=sc[:], func=EXPF, scale=SCALE)
                    nc.tensor.matmul(pvE[:], v_tiles[:, t, g, :], ex[:, 0, :],
                                     start=(t == 0), stop=(t == TT - 1))
                    nc.tensor.matmul(pvO[:], v_tiles[:, t, g, :], ex[:, 1, :],
                                     start=(t == 0), stop=(t == TT - 1))
                # normalize by Z (row D of pv psums) and write attn_sb
                with nc.allow_low_precision(reason="f32r reciprocal feeds f32r matmul"):
                    nc.vector.reciprocal(zrec[0:1, :], pvE[D:D + 1, :])
                    nc.vector.reciprocal(zrec[32:33, :], pvO[D:D + 1, :])
                rbcp = ps2.tile([P, CH], f32, tag="aux", bufs=2)
                nc.tensor.matmul(rbcp[:], bsel[:], zrec[:], start=True, stop=True)
                rbc = work.tile([P, CH], f32, tag="rbc")
                nc.vector.tensor_copy(rbc[:], rbcp[:])
                nc.vector.tensor_mul(attn_sb[0:D, p, cs], pvE[0:D, :], rbc[0:D, :])
                nc.vector.tensor_mul(attn_sb[D:P, p, cs], pvO[0:D, :], rbc[D:P, :])
                # previous chunk's o-projection, one s-tile per pair
                if c > 0:
                    emit_oproj((c - 1) * (CH // P) + p)
        for st in range((NCH - 1) * (CH // P), NCH * (CH // P)):
            emit_oproj(st)

    for pool in (expp, work, persist, htp, wkvp, wbig, consts):
        pool.release()


_NC_CACHE = None


def build_nc():
    global _NC_CACHE
    if _NC_CACHE is None:
        nc = bacc.Bacc("TRN2")
        with tile.TileContext(nc) as tc:
            _emit(tc)
        nc.compile()
        _NC_CACHE = nc
    return _NC_CACHE


def _bsel_np():
    b = np.zeros((33, P), dtype=np.float32)
    b[0, 0:D] = 1.0
    b[32, D:P] = 1.0
    return b


def make_in_maps(hidden_state, Wq, bq, Wk, bk, Wv, bv, Wo):
    hidden_state = np.asarray(hidden_state, dtype=np.float32)
    Wq, Wk, Wv, Wo = (np.asarray(a, dtype=np.float32) for a in (Wq, Wk, Wv, Wo))
    bq, bk, bv = (np.asarray(a, dtype=np.float32) for a in (bq, bk, bv))
    htb = [np.ascontiguousarray(hidden_state[b].T) for b in range(B)]
    ones = np.ones((P, 32), dtype=np.float32)
    in_maps = []
    for c in range(NCORES):
        b, gs = divmod(c, GS)
        in_maps.append({
            "ht": htb[b],
            "ones": np.ones((P, D), dtype=np.float32),
            "bsel": _bsel_np(),
            "zeros": np.zeros((33, CH), dtype=np.float32),
            "wq": np.ascontiguousarray(Wq[gs * DQ:(gs + 1) * DQ, :].T),
            "wk": np.ascontiguousarray(Wk[gs * DKV:(gs + 1) * DKV, :].T),
            "wv": np.ascontiguousarray(Wv[gs * DKV:(gs + 1) * DKV, :].T),
            "wo": np.ascontiguousarray(Wo[:, gs * DQ:(gs + 1) * DQ].T),
            "bq": np.ascontiguousarray(bq[gs * DQ:(gs + 1) * DQ]),
            "bk": np.ascontiguousarray(bk[gs * DKV:(gs + 1) * DKV]),
            "bv": np.ascontiguousarray(bv[gs * DKV:(gs + 1) * DKV]),
        })
    return in_maps


def unshard(results, bo):
    bo = np.asarray(bo, dtype=np.float32)
    out = np.empty((B, S, HID), dtype=np.float32)
    for b in range(B):
        acc = np.zeros((S, HID), dtype=np.float64)
        for gs in range(GS):
            acc += results[b * GS + gs]["opart"]
        out[b] = (acc + bo).astype(np.float32)
    return out


def kernel(hidden_state, attention_mask, Wq, bq, Wk, bk, Wv, bv, Wo, bo):
    # attention_mask is all-ones for this problem (fill: ones) -> identity.
    nc = build_nc()
    in_maps = make_in_maps(hidden_state, Wq, bq, Wk, bk, Wv, bv, Wo)
    res = run_bass_kernel_spmd(nc, in_maps, list(range(NCORES)))
    return unshard(res.results, bo)



# revision 9
# speedup vs baseline: 1.2050x; 1.2050x over previous
"""GroupedQueryAttention Trainium2 kernel (v2).

Sharding: 8 cores = 2 (batch) x 4 (KV-head groups). Each core handles one
batch and 2 KV heads (8 query heads, DQ=512 q dims, DKV=128 kv dims).

Per-core pipeline (CoreSim matmul cost = out_cols x cycles_per_row; bf16 is
1.0, fp8+DoubleRow 0.5 with 2x contraction per instruction):
  - projections: qT (prescaled), k, v
  - QK^T per head into 2-bank psum "duos" [128t, 2, 512s] (bf16)
  - exp split: Activation engine (exact exp) + DVE (exp2 bit-trick)
  - PV in [s, d] orientation (16x fewer streamed cols than [d, s]):
    lhsT = ex duo slice, rhs = v tiles with a ones column -> Z lands in col 64
  - normalize on s-partitions (DVE reciprocal + broadcast mult)
  - DMA-transpose attn [s,d] -> attnT [d,s] (XBAR crossbar, no PE cost)
  - o-proj row-parallel; host sums the 4 partials per batch and adds bo.

PSUM: "sc" tag [128,2,512] x3 slots (6 banks; score duos AND PV half-heads
rotate through it) + "b1" tag [128,512] x2 (proj/o-proj) = 8 banks.
"""

import numpy as np
import ml_dtypes

import concourse.bass as bass
import concourse.mybir as mybir
import concourse.tile as tile
from concourse import bacc
from concourse.bass_utils import run_bass_kernel_spmd

# ---- problem dims ----
P = 128
B, S, HID = 2, 2048, 2048
NH, G = 32, 8
HG = NH // G            # 4 query heads per KV head
D = HID // NH           # 64
NCORES = 8
GS = NCORES // B        # 4 head-group shards
DQ = HID // GS          # 512 q dims per core
DKV = G * D // GS       # 128 kv dims per core (2 KV heads)
CH = 512                # s-chunk width
NCH = S // CH           # 4
KT = HID // P           # 16 contraction tiles (bf16 proj)
NDR = KT // 2           # 8 DoubleRow contraction tiles (fp8 proj)
TT = S // P             # 16 key tiles
NHEADS = 8              # query heads per core
NMT = DQ // P           # 4 q-proj output tiles

# ---- config flags (accuracy-gated) ----
PROJ_F8 = False         # fp8 DoubleRow projections
W_F8 = False            # fp8 exp weights + fp8 v -> PV DoubleRow
AT_F8 = False           # fp8 attn + Wo -> o-proj DoubleRow
TRICK_PER16 = 0         # duos per 16 routed to DVE exp2 bit-trick (0=Act only)

f32 = mybir.dt.float32
bf16 = mybir.dt.bfloat16
f8 = mybir.dt.float8e4
i32 = mybir.dt.int32
EXPF = mybir.ActivationFunctionType.Exp
DR = mybir.MatmulPerfMode.DoubleRow
ADD = mybir.AluOpType.add
MULT = mybir.AluOpType.mult

NP_BF16 = ml_dtypes.bfloat16
NP_F8 = ml_dtypes.float8_e4m3

SCALE = 1.0 / float(np.sqrt(D))
POW_N = 16384.0                       # act exp scale (scores pre-scaled by SCALE/POW_N)
QPRE = SCALE / POW_N
WSCALE = 2.0 ** -8                    # keeps exp weights under fp8e4m3 max
LNW = float(np.log(WSCALE))
LOG2E = float(np.log2(np.e))
# exp2 bit-trick: i = x*POW_N*log2e*2^23 + (127 + log2(WSCALE) - corr)*2^23
TRICK_K = POW_N * LOG2E * (2.0 ** 23)
TRICK_B = float((127.0 + np.log2(WSCALE) - np.log2(1.0443)) * (2.0 ** 23))

W_DT = f8 if W_F8 else bf16


def _emit(tc):
    nc = tc.nc

    # ---- DRAM ----
    if PROJ_F8:
        ht_d = nc.dram_tensor("ht", [NCH, NDR, P, 2, CH], f8, kind="ExternalInput")
        wq_d = nc.dram_tensor("wq", [P, NDR, 2, DQ], f8, kind="ExternalInput")
        wk_d = nc.dram_tensor("wk", [P, NDR, 2, DKV], f8, kind="ExternalInput")
        wv_d = nc.dram_tensor("wv", [P, NDR, 2, DKV], f8, kind="ExternalInput")
    else:
        ht_d = nc.dram_tensor("ht", [NCH, KT, P, CH], bf16, kind="ExternalInput")
        wq_d = nc.dram_tensor("wq", [P, KT, DQ], bf16, kind="ExternalInput")
        wk_d = nc.dram_tensor("wk", [P, KT, DKV], bf16, kind="ExternalInput")
        wv_d = nc.dram_tensor("wv", [P, KT, DKV], bf16, kind="ExternalInput")
    if AT_F8:
        wo_d = nc.dram_tensor("wo", [P, 2, 2, HID], f8, kind="ExternalInput")
    else:
        wo_d = nc.dram_tensor("wo", [P, NMT, HID], bf16, kind="ExternalInput")
    bq_d = nc.dram_tensor("bq", [P, NMT], f32, kind="ExternalInput")  # pre x QPRE
    bk_d = nc.dram_tensor("bk", [P, 1], f32, kind="ExternalInput")
    bv_d = nc.dram_tensor("bv", [P, 1], f32, kind="ExternalInput")
    opart = nc.dram_tensor("opart", [S, HID], bf16, kind="ExternalOutput")

    # ---- SBUF pools ----
    consts = tc.alloc_tile_pool(name="consts", bufs=1)
    wpool = tc.alloc_tile_pool(name="wpool", bufs=1)
    htp = tc.alloc_tile_pool(name="htp", bufs=4)
    persist = tc.alloc_tile_pool(name="persist", bufs=1)
    expool = tc.alloc_tile_pool(name="expool", bufs=1)
    work = tc.alloc_tile_pool(name="work", bufs=1)

    bq_t = consts.tile([P, NMT], f32)
    nc.sync.dma_start(out=bq_t[:], in_=bq_d[:])
    bk_t = consts.tile([P, 1], f32)
    nc.sync.dma_start(out=bk_t[:], in_=bk_d[:])
    bv_t = consts.tile([P, 1], f32)
    nc.sync.dma_start(out=bv_t[:], in_=bv_d[:])
    lnw_t = consts.tile([P, 1], f32)
    nc.gpsimd.memset(lnw_t[:], LNW)

    if PROJ_F8:
        wq_sb = wpool.tile([P, NDR, 2, DQ], f8)
        wk_sb = wpool.tile([P, NDR, 2, DKV], f8)
        wv_sb = wpool.tile([P, NDR, 2, DKV], f8)
    else:
        wq_sb = wpool.tile([P, KT, DQ], bf16)
        wk_sb = wpool.tile([P, KT, DKV], bf16)
        wv_sb = wpool.tile([P, KT, DKV], bf16)
    nc.sync.dma_start(out=wk_sb[:], in_=wk_d[:])
    nc.sync.dma_start(out=wv_sb[:], in_=wv_d[:])
    nc.sync.dma_start(out=wq_sb[:], in_=wq_d[:])
    if AT_F8:
        wo_sb = wpool.tile([P, 2, 2, HID], f8)
    else:
        wo_sb = wpool.tile([P, NMT, HID], bf16)
    nc.sync.dma_start(out=wo_sb[:], in_=wo_d[:])

    # persistent activations
    qT_sb = persist.tile([P, NMT, S], bf16)        # prescaled q: [dpair, pair, s]
    ktrepA = persist.tile([P, S], bf16)            # kv head 0 on both halves
    ktrepB = persist.tile([P, S], bf16)            # kv head 1 on both halves
    k_sb = persist.tile([P, S], bf16)
    if W_F8:
        v_dr = persist.tile([P, NDR, 2, 2, 65], f8)   # [t, j, i(plane), g, dv|1]
        nc.gpsimd.memset(v_dr[:, :, :, :, 64:65], 1.0)
    else:
        v_nd = persist.tile([P, TT, 2, 65], bf16)     # [t, tt, g, dv|1]
        nc.gpsimd.memset(v_nd[:, :, :, 64:65], 1.0)
    attn_nrm = [persist.tile([P, 4, NHEADS, D], bf16, name=f"anrm{i}")
                for i in range(2)]

    ht_tiles = {}
    attnT = {}
    state = {"duo": 0, "misc": [], "pv": [], "epi": [], "credit": 0.0}

    def misc_defer(cost_ns, fn):
        state["misc"].append((cost_ns, fn))

    def drain_misc(credit_ns):
        state["credit"] += credit_ns
        while state["misc"] and state["credit"] > 0:
            cost, fn = state["misc"].pop(0)
            fn()
            state["credit"] -= cost

    def pop_pv():
        if state["pv"]:
            state["pv"].pop(0)()
        elif state["epi"]:
            state["epi"].pop(0)()

    def flush_all():
        while state["pv"]:
            state["pv"].pop(0)()
        while state["epi"]:
            state["epi"].pop(0)()
        while state["misc"]:
            state["misc"].pop(0)[1]()

    with tc.tile_pool(name="ps_sc", bufs=3, space="PSUM") as ps_sc, \
         tc.tile_pool(name="ps_b1", bufs=2, space="PSUM") as ps_b1:

        def load_ht(c):
            if PROJ_F8:
                htt = htp.tile([P, NDR, 2, CH], f8, tag="ht", name=f"ht{c}")
                nc.sync.dma_start(out=htt[:],
                                  in_=ht_d[c].rearrange("kt p i s -> p kt i s"))
            else:
                htt = htp.tile([P, KT, CH], bf16, tag="ht", name=f"ht{c}")
                nc.sync.dma_start(out=htt[:],
                                  in_=ht_d[c].rearrange("kt p s -> p kt s"))
            ht_tiles[c] = htt

        def proj_mm(out_ap, w_sb, mcols, c):
            htt = ht_tiles[c]
            if PROJ_F8:
                for kt in range(NDR):
                    nc.tensor.matmul(out_ap, w_sb[:, kt, :, mcols], htt[:, kt, :, :],
                                     start=(kt == 0), stop=(kt == NDR - 1),
                                     perf_mode=DR)
            else:
                for kt in range(KT):
                    nc.tensor.matmul(out_ap, w_sb[:, kt, mcols], htt[:, kt, :],
                                     start=(kt == 0), stop=(kt == KT - 1))

        def emit_q_proj_mt(c, mt):
            cs = slice(c * CH, (c + 1) * CH)
            b1 = ps_b1.tile([P, CH], f32, tag="b1", name=f"qp{c}_{mt}")
            proj_mm(b1[:], wq_sb, slice(mt * P, (mt + 1) * P), c)
            nc.vector.tensor_scalar(out=qT_sb[:, mt, cs], in0=b1[:],
                                    scalar1=QPRE, scalar2=bq_t[:, mt:mt + 1],
                                    op0=MULT, op1=ADD)

        def emit_k_proj(c):
            cs = slice(c * CH, (c + 1) * CH)
            b1 = ps_b1.tile([P, CH], f32, tag="b1", name=f"kp{c}")
            proj_mm(b1[:], wk_sb, slice(0, DKV), c)
            nc.vector.tensor_scalar_add(k_sb[:, cs], b1[:], bk_t[:, 0:1])
            # duplicate each kv head onto both partition halves for paired QK
            nc.sync.dma_start(out=ktrepA[0:D, cs], in_=k_sb[0:D, cs])
            nc.sync.dma_start(out=ktrepA[D:P, cs], in_=k_sb[0:D, cs])
            nc.sync.dma_start(out=ktrepB[0:D, cs], in_=k_sb[D:P, cs])
            nc.sync.dma_start(out=ktrepB[D:P, cs], in_=k_sb[D:P, cs])

        def emit_v_proj(c):
            b1 = ps_b1.tile([P, CH], f32, tag="b1", name=f"vp{c}")
            proj_mm(b1[:], wv_sb, slice(0, DKV), c)
            vstage = work.tile([P, CH], bf16, tag="vstage", bufs=2, name=f"vs{c}")
            nc.vector.tensor_scalar_add(vstage[:], b1[:], bv_t[:, 0:1])
            vtr = work.tile([P, 4, P], bf16, tag="vtr", bufs=2, name=f"vtr{c}")
            nc.sync.dma_start_transpose(vtr[:], vstage[:])   # [t, tt, dkv]
            for g in range(2):
                gsl = slice(g * D, (g + 1) * D)
                if W_F8:
                    for jj in range(2):
                        j = 2 * c + jj
                        nc.vector.tensor_copy(v_dr[:, j, :, g, 0:D],
                                              vtr[:, 2 * jj:2 * jj + 2, gsl])
                else:
                    nc.vector.tensor_copy(v_nd[:, 4 * c:4 * (c + 1), g, 0:D],
                                          vtr[:, :, gsl])

        def emit_exp(duo, ex):
            i = state["duo"]
            state["duo"] += 1
            if (i % 16) < TRICK_PER16:
                tk = work.tile([P, 2, CH], i32, tag="trick", bufs=2, name="tk")
                nc.vector.tensor_scalar(out=tk[:], in0=duo[:], scalar1=TRICK_K,
                                        scalar2=TRICK_B, op0=MULT, op1=ADD)
                nc.vector.tensor_copy(ex[:], tk[:].bitcast(f32))
            else:
                nc.scalar.activation(out=ex[:], in_=duo[:], func=EXPF,
                                     scale=POW_N, bias=lnw_t[:])

        def emit_pv_half(c, h, half, ex_tiles):
            g = h // 4
            pv = ps_sc.tile([P, 2, CH], f32, tag="sc", name=f"pv{c}_{h}_{half}")
            for sl in range(2):
                st = 2 * half + sl
                ss = slice(st * P, (st + 1) * P)
                if W_F8:
                    for j in range(NDR):
                        nc.tensor.matmul(pv[:, sl, 0:65], ex_tiles[j][:, :, ss],
                                         v_dr[:, j, :, g, :],
                                         start=(j == 0), stop=(j == NDR - 1),
                                         perf_mode=DR)
                else:
                    for t in range(TT):
                        nc.tensor.matmul(pv[:, sl, 0:65],
                                         ex_tiles[t // 2][:, t % 2, ss],
                                         v_nd[:, t, g, :],
                                         start=(t == 0), stop=(t == TT - 1))
            zr = work.tile([P, 2, 1], f32, tag="zr", bufs=3, name="zr")
            nc.vector.reciprocal(zr[:], pv[:, :, 64:65])
            nc.vector.tensor_tensor(
                out=attn_nrm[c % 2][:, 2 * half:2 * half + 2, h, :],
                in0=pv[:, :, 0:D], in1=zr[:].broadcast_to((P, 2, D)), op=MULT)

        def emit_attnT(c, st):
            at = work.tile([P, NMT, P], bf16, tag="attnT", bufs=6,
                           name=f"at{c}_{st}")
            nc.sync.dma_start_transpose(at[:], attn_nrm[c % 2][:, st, :, :])
            if AT_F8:
                at8 = work.tile([P, NMT, P], f8, tag="attnT8", bufs=10,
                                name=f"at8_{c}_{st}")
                nc.gpsimd.tensor_copy(at8[:], at[:])
                attnT[(c, st)] = at8
            else:
                attnT[(c, st)] = at

        def emit_oproj(c, st, hc):
            at = attnT.pop((c, st)) if hc == NMT - 1 else attnT[(c, st)]
            ss = slice((c * 4 + st) * P, (c * 4 + st + 1) * P)
            hs = slice(hc * CH, (hc + 1) * CH)
            b1 = ps_b1.tile([P, CH], f32, tag="b1", name=f"op{c}_{st}_{hc}")
            if AT_F8:
                for j in range(2):
                    nc.tensor.matmul(b1[:], at[:, 2 * j:2 * j + 2, :],
                                     wo_sb[:, j, :, hs],
                                     start=(j == 0), stop=(j == 1), perf_mode=DR)
            else:
                for kt in range(NMT):
                    nc.tensor.matmul(b1[:], at[:, kt, :], wo_sb[:, kt, hs],
                                     start=(kt == 0), stop=(kt == NMT - 1))
            ostg = work.tile([P, CH], bf16, tag="ostg", bufs=4, name="ostg")
            nc.vector.tensor_copy(ostg[:], b1[:])
            nc.sync.dma_start(out=opart[ss, hs], in_=ostg[:])

        # ---------- prologue: all k/v projections + q chunk 0 ----------
        load_ht(0)
        load_ht(1)
        emit_k_proj(0)
        emit_v_proj(0)
        load_ht(2)
        emit_k_proj(1)
        emit_v_proj(1)
        load_ht(3)
        emit_k_proj(2)
        emit_v_proj(2)
        emit_k_proj(3)
        emit_v_proj(3)
        for mt in range(NMT):
            emit_q_proj_mt(0, mt)
        for c in (1, 2, 3):
            for mt in range(NMT):
                misc_defer(3400, (lambda cc, m: lambda: emit_q_proj_mt(cc, m))(c, mt))

        # ---------- main loop ----------
        for c in range(NCH):
            cs = slice(c * CH, (c + 1) * CH)
            for h in range(NHEADS):
                pair, e = h // 2, h % 2
                ktrep = ktrepA if h < 4 else ktrepB
                erange = slice(e * D, (e + 1) * D)
                ex_tiles = []
                for j2 in range(NDR):
                    duo = ps_sc.tile([P, 2, CH], f32, tag="sc", name="duo")
                    for i2 in range(2):
                        ts_ = slice((2 * j2 + i2) * P, (2 * j2 + i2 + 1) * P)
                        nc.tensor.matmul(duo[:, i2, :], ktrep[erange, ts_],
                                         qT_sb[erange, pair, cs],
                                         tile_position=(e * D, 0),
                                         start=True, stop=True)
                    ex = expool.tile([P, 2, CH], W_DT, tag="ex", bufs=12, name="ex")
                    emit_exp(duo, ex)
                    ex_tiles.append(ex)
                    pop_pv()
                    drain_misc(430)
                for half in range(2):
                    state["pv"].append(
                        (lambda cc, hh, hf, exs:
                         lambda: emit_pv_half(cc, hh, hf, exs))(c, h, half, ex_tiles))
            # chunk epilogue: pops only when the PV queue is empty, which
            # keeps attnT after this chunk's last PV halves
            for st in range(4):
                state["epi"].append(
                    (lambda cc, s_: lambda: emit_attnT(cc, s_))(c, st))
            for st in range(4):
                for hc in range(NMT):
                    state["epi"].append(
                        (lambda cc, s_, hh: lambda: emit_oproj(cc, s_, hh))
                        (c, st, hc))
        flush_all()

    for pool in (work, expool, persist, htp, wpool, consts):
        pool.release()


_NC_CACHE = None


def build_nc():
    global _NC_CACHE
    if _NC_CACHE is None:
        nc = bacc.Bacc("TRN2")
        with tile.TileContext(nc) as tc:
            _emit(tc)
        nc.compile()
        _NC_CACHE = nc
    return _NC_CACHE


def _pack_dr_w(Wslice):
    # Wslice [M, HID] (torch layout) -> [P, NDR, 2, M] fp8, 256-deep planes
    M = Wslice.shape[0]
    w = Wslice.T.reshape(NDR, 2, P, M)          # [kt, i, p, m]
    return np.ascontiguousarray(w.transpose(2, 0, 1, 3)).astype(NP_F8)


def make_in_maps(hidden_state, Wq, bq, Wk, bk, Wv, bv, Wo):
    hidden_state = np.asarray(hidden_state, np.float32)
    Wq, Wk, Wv, Wo = (np.asarray(a, np.float32) for a in (Wq, Wk, Wv, Wo))
    bq, bk, bv = (np.asarray(a, np.float32) for a in (bq, bk, bv))

    hts = []
    for b in range(B):
        htb = hidden_state[b].T                  # [HID, S]
        if PROJ_F8:
            h4 = htb.reshape(NDR, 2, P, NCH, CH)  # [kt, i, p, c, s]
            hts.append(np.ascontiguousarray(
                h4.transpose(3, 0, 2, 1, 4)).astype(NP_F8))
        else:
            h4 = htb.reshape(KT, P, NCH, CH)
            hts.append(np.ascontiguousarray(
                h4.transpose(2, 0, 1, 3)).astype(NP_BF16))

    in_maps = []
    for core in range(NCORES):
        b, gs = divmod(core, GS)
        wq_s = Wq[gs * DQ:(gs + 1) * DQ, :]       # [DQ, HID]
        wk_s = Wk[gs * DKV:(gs + 1) * DKV, :]
        wv_s = Wv[gs * DKV:(gs + 1) * DKV, :]
        wo_s = Wo[:, gs * DQ:(gs + 1) * DQ]       # [HID, DQ]
        if PROJ_F8:
            wq_p, wk_p, wv_p = _pack_dr_w(wq_s), _pack_dr_w(wk_s), _pack_dr_w(wv_s)
        else:
            wq_p = np.ascontiguousarray(
                wq_s.T.reshape(KT, P, DQ).transpose(1, 0, 2)).astype(NP_BF16)
            wk_p = np.ascontiguousarray(
                wk_s.T.reshape(KT, P, DKV).transpose(1, 0, 2)).astype(NP_BF16)
            wv_p = np.ascontiguousarray(
                wv_s.T.reshape(KT, P, DKV).transpose(1, 0, 2)).astype(NP_BF16)
        if AT_F8:
            wo_p = np.ascontiguousarray(
                wo_s.T.reshape(2, 2, P, HID).transpose(2, 0, 1, 3)).astype(NP_F8)
        else:
            wo_p = np.ascontiguousarray(
                wo_s.T.reshape(NMT, P, HID).transpose(1, 0, 2)).astype(NP_BF16)
        in_maps.append({
            "ht": hts[b],
            "wq": wq_p, "wk": wk_p, "wv": wv_p, "wo": wo_p,
            "bq": np.ascontiguousarray(
                (bq[gs * DQ:(gs + 1) * DQ] * QPRE).reshape(NMT, P).T
            ).astype(np.float32),
            "bk": bk[gs * DKV:(gs + 1) * DKV].reshape(P, 1).astype(np.float32),
            "bv": bv[gs * DKV:(gs + 1) * DKV].reshape(P, 1).astype(np.float32),
        })
    return in_maps


def unshard(results, bo):
    bo = np.asarray(bo, np.float32)
    out = np.empty((B, S, HID), np.float32)
    for b in range(B):
        acc = np.zeros((S, HID), np.float64)
        for gs in range(GS):
            acc += results[b * GS + gs]["opart"].astype(np.float32)
        out[b] = (acc + bo).astype(np.float32)
    return out


def kernel(hidden_state, attention_mask, Wq, bq, Wk, bk, Wv, bv, Wo, bo):
    # attention_mask is all-ones for this problem -> identity.
    nc = build_nc()
    in_maps = make_in_maps(hidden_state, Wq, bq, Wk, bk, Wv, bv, Wo)
    res = run_bass_kernel_spmd(nc, in_maps, list(range(NCORES)))
    return unshard(res.results, bo)
